# revision 6
# baseline (speedup 1.0000x reference)
"""Trainium2 Bass kernel for ConditionedPNA (3-layer PNAConv, N=50000, D=128, DEG=12).

Sharding/strategy (8 NeuronCores, SPMD):
  - Nodes sharded N/8 per core (NP=6272 padded). Row indices are sorted
    (repeat(arange(N), DEG)) with uniform degree 12, so each node's edges are
    contiguous and colocated; segment reductions are purely local.
  - Algebra: m_e = A[row_e] + B[col_e] + bpre with A = W1@h, B = W2@h.
    mean/max/min reduce to (A + bpre) + stats of B[col]; std = std of B[col].
    The A term is folded into an effective Whh on the host
    (Whh_eff = Wlin@Wh + (Weff_mean+Weff_max+Weff_min)@W1), bpre/bpost/blin
    into one output bias, the degree scalers (deg==12) into Weff, and the
    mean's 1/12 into Weff_mean. The residual is added on-chip in f32.
  - Per layer: B shard (node-major bf16) -> AllGather into a DRAM table laid
    out as 8 blocks of (1 zero row + NP rows) so that both halves of the
    int16-indexed gather have a zeros row at index 0 of their view.
  - Gather: dma_gather (SWDGE, transpose mode) fetches each 4-tile chunk's
    6144 edge rows feature-major in 2 instructions (lo = table blocks 0-4,
    hi = blocks 5-7; positions belonging to the other half point at the
    zero row), merged with one add. This replaces 48 indirect row-DMAs
    (~1us of GPSIMD time each) per chunk.
  - Aggregation: per 128-node tile the merged (128f, 128n, 12k) view is
    reduced over k with single tensor_reduce ops (sum/max/min + square+sum),
    var/std in f32, everything already feature-major for the post-GEMM:
    h'^T = Whh_eff.T@h^T + sum_j WeffT_j.T@agg_j^T + bias; residual via DVE.
"""
import math
import numpy as np
import ml_dtypes

import concourse.bass as bass
from concourse import bacc
import concourse.tile as tile
from concourse import mybir
from concourse.bass_utils import run_bass_kernel_spmd

D, DEG, L, CORES = 128, 12, 3, 8
NQ = 4  # SWDGE queues
CT = 4  # tiles per gather chunk
F32 = mybir.dt.float32
BF16 = mybir.dt.bfloat16
I16 = mybir.dt.int16
LOB = 5  # table blocks (shards) reachable by the lo gather

_hist = np.array([1.0] * 10 + [2.0] * 10)
AVG_LOG = float((np.log(np.arange(20) + 1.0) * _hist).sum() / _hist.sum())


class Cfg:
    def __init__(self, n):
        self.N = n
        self.NC = n // CORES
        self.NP = ((self.NC + 127) // 128) * 128
        self.TILES = self.NP // 128
        self.CHUNKS = (self.TILES + CT - 1) // CT
        self.W = CT * 128 * DEG // 16  # idx slots per partition per chunk


def build(cfg, repeat=1):
    A = mybir.AluOpType
    AF = mybir.ActivationFunctionType
    AX = mybir.AxisListType
    NP, TILES, CHUNKS, W = cfg.NP, cfg.TILES, cfg.CHUNKS, cfg.W
    nc = bacc.Bacc("TRN2", target_bir_lowering=False, num_devices=CORES,
                   num_swdge_queues=NQ)

    xT = nc.dram_tensor("xT", [D, NP], F32, kind="ExternalInput")
    idxw = nc.dram_tensor("idxw", [128, CHUNKS, 2, W], I16, kind="ExternalInput")
    wpack = nc.dram_tensor("wpack", [D, L, 6, D], BF16, kind="ExternalInput")
    bpack = nc.dram_tensor("bpack", [D, L], F32, kind="ExternalInput")
    outT = nc.dram_tensor("outT", [D, NP], F32, kind="ExternalOutput")

    agin = nc.dram_tensor("agin", [NP, D], BF16)
    tabl = nc.dram_tensor("tabl", [CORES * NP, D], BF16, addr_space="Shared")

    with tile.TileContext(nc) as tc:
        with (
            tc.tile_pool(name="persist", bufs=1) as pp,
            tc.tile_pool(name="gat", bufs=2) as gp,
            tc.tile_pool(name="small", bufs=2) as sp,
            tc.tile_pool(name="psum", bufs=2, space="PSUM") as ps,
        ):
            hT_a = pp.tile([D, NP], F32)
            hT_b = pp.tile([D, NP], F32)
            hT = [hT_a, hT_b]
            Wt = pp.tile([D, L, 6, D], BF16)
            nc.sync.dma_start(out=Wt[:], in_=wpack[:])
            Bv = pp.tile([D, L], F32)
            nc.sync.dma_start(out=Bv[:], in_=bpack[:])
            IX = pp.tile([128, CHUNKS, 2, W], I16)
            nc.sync.dma_start(out=IX[:], in_=idxw[:])
            eps = pp.tile([D, 1], F32)
            nc.vector.memset(eps[:], 1e-5)
            nc.sync.dma_start(out=hT[0][:], in_=xT[:])

            # lo gather sees the whole table; hi gather the last 3 shards.
            # "other half" positions point at a pad row (zeros) of the view.
            tabl_lo = tabl[:]
            tabl_hi = tabl[LOB * NP:, :]

            for ll in range(repeat * L):
                l = ll % L
                hin = hT[ll % 2]
                hout = hT[(ll + 1) % 2]
                w2t = Wt[:, l, 0, :]
                whh = Wt[:, l, 1, :]
                weff = [Wt[:, l, 2 + j, :] for j in range(4)]
                bout = Bv[:, l:l + 1]

                # ---- B shard (node-major bf16) -> agin -> AllGather ----
                for t in range(TILES):
                    hbt = sp.tile([D, 128], BF16, tag="hbt")
                    nc.scalar.activation(hbt[:], hin[:, t * 128:(t + 1) * 128],
                                         AF.Copy)
                    bp = ps.tile([128, D], F32, space="PSUM", tag="bp")
                    nc.tensor.matmul(out=bp[:], lhsT=hbt[:], rhs=w2t,
                                     start=True, stop=True)
                    bs = sp.tile([128, D], BF16, tag="bs")
                    nc.scalar.activation(bs[:], bp[:], AF.Copy)
                    nc.sync.dma_start(out=agin[t * 128:(t + 1) * 128, :], in_=bs[:])
                nc.gpsimd.collective_compute(
                    "AllGather", A.bypass,
                    replica_groups=[list(range(CORES))],
                    ins=[agin[:]], outs=[tabl[:]],
                )

                # ---- gather + aggregate + post-GEMM, per chunk of CT tiles ----
                for c in range(CHUNKS):
                    tl = min(CT, TILES - c * CT)  # tiles in this chunk
                    nidx = tl * 128 * DEG
                    w = nidx // 16
                    glo = gp.tile([128, CT * 128, DEG], BF16, tag="glo")
                    ghi = gp.tile([128, CT * 128, DEG], BF16, tag="ghi")
                    lo3 = glo[:, :tl * 128, :].rearrange("p a b -> p (a b)") \
                        .unsqueeze(1)
                    hi3 = ghi[:, :tl * 128, :].rearrange("p a b -> p (a b)") \
                        .unsqueeze(1)
                    nc.gpsimd.dma_gather(
                        out_ap=lo3, in_ap=tabl_lo, idxs_ap=IX[:, c, 0, :w],
                        num_idxs=nidx, num_idxs_reg=nidx, elem_size=D,
                        transpose=True, single_packet=False,
                        queue_num=0)
                    nc.gpsimd.dma_gather(
                        out_ap=hi3, in_ap=tabl_hi, idxs_ap=IX[:, c, 1, :w],
                        num_idxs=nidx, num_idxs_reg=nidx, elem_size=D,
                        transpose=True, single_packet=False,
                        queue_num=0)
                    gm = gp.tile([128, CT * 128, DEG], BF16, tag="gm")
                    nc.gpsimd.tensor_tensor(out=gm[:, :tl * 128, :],
                                            in0=glo[:, :tl * 128, :],
                                            in1=ghi[:, :tl * 128, :], op=A.add)

                    for ti in range(tl):
                        t = c * CT + ti
                        tv = gm[:, ti * 128:(ti + 1) * 128, :]  # (128f,128n,12)
                        S = sp.tile([128, 128], F32, tag="S")
                        nc.vector.tensor_reduce(out=S[:], in_=tv, axis=AX.X,
                                                op=A.add)
                        MX = sp.tile([128, 128], BF16, tag="MX")
                        nc.vector.tensor_reduce(out=MX[:], in_=tv, axis=AX.X,
                                                op=A.max)
                        MN = sp.tile([128, 128], BF16, tag="MN")
                        nc.vector.tensor_reduce(out=MN[:], in_=tv, axis=AX.X,
                                                op=A.min)
                        g2 = sp.tile([128, 128, DEG], BF16, tag="g2")
                        nc.scalar.square(g2[:], tv)
                        S2 = sp.tile([128, 128], F32, tag="S2")
                        nc.vector.tensor_reduce(out=S2[:], in_=g2[:], axis=AX.X,
                                                op=A.add)
                        m2 = sp.tile([128, 128], F32, tag="m2")
                        nc.vector.tensor_tensor(out=m2[:], in0=S[:], in1=S[:],
                                                op=A.mult)
                        nc.vector.tensor_scalar_mul(m2[:], m2[:], 1.0 / 144.0)
                        var = sp.tile([128, 128], F32, tag="var")
                        nc.vector.scalar_tensor_tensor(
                            out=var[:], in0=S2[:], scalar=1.0 / DEG, in1=m2[:],
                            op0=A.mult, op1=A.subtract)
                        nc.vector.tensor_scalar_max(var[:], var[:], 0.0)
                        STD = sp.tile([128, 128], BF16, tag="STD")
                        nc.scalar.activation(STD[:], var[:], AF.Sqrt, bias=eps[:])
                        Sb = sp.tile([128, 128], BF16, tag="Sb")
                        nc.scalar.activation(Sb[:], S[:], AF.Copy)

                        hslab = hin[:, t * 128:(t + 1) * 128]
                        hbt2 = sp.tile([D, 128], BF16, tag="hbt2")
                        nc.scalar.activation(hbt2[:], hslab, AF.Copy)
                        hp = ps.tile([128, 128], F32, space="PSUM", tag="hp")
                        nc.tensor.matmul(out=hp[:], lhsT=whh, rhs=hbt2[:],
                                         start=True, stop=False)
                        for j, part in enumerate([Sb, MX, MN, STD]):
                            nc.tensor.matmul(out=hp[:], lhsT=weff[j],
                                             rhs=part[:], start=False,
                                             stop=(j == 3))
                        ho = sp.tile([D, 128], F32, tag="ho")
                        nc.scalar.activation(ho[:], hp[:], AF.Identity,
                                             bias=bout)
                        nc.vector.tensor_tensor(
                            out=hout[:, t * 128:(t + 1) * 128], in0=ho[:],
                            in1=hslab, op=A.add)
                if cfg.NC < NP:
                    nc.vector.memset(hout[:, cfg.NC:NP], 0.0)

            nc.sync.dma_start(out=outT[:], in_=hT[(repeat * L) % 2][:])
    nc.compile()
    return nc


def prep_inputs(cfg, x, edge_index, Wpre, bpre, Wpost, bpost, Wlin, blin):
    x = np.asarray(x, np.float32)
    ei = np.asarray(edge_index)
    Wpre = np.asarray(Wpre, np.float64)
    bpre = np.asarray(bpre, np.float64)
    Wpost = np.asarray(Wpost, np.float64)
    bpost = np.asarray(bpost, np.float64)
    Wlin = np.asarray(Wlin, np.float64)
    blin = np.asarray(blin, np.float64)
    N, NC, NP = cfg.N, cfg.NC, cfg.NP
    TILES, CHUNKS, W = cfg.TILES, cfg.CHUNKS, cfg.W

    row = ei[0].astype(np.int64)
    col = ei[1].astype(np.int64)
    assert (row == np.repeat(np.arange(N), DEG)).all(), \
        "kernel assumes sorted rows, uniform degree"
    dlog = math.log(DEG + 1.0)
    k1 = dlog / AVG_LOG
    k2 = AVG_LOG / dlog

    wpack = np.zeros((D, L, 6, D), np.float32)
    bpack = np.zeros((D, L), np.float32)
    for l in range(L):
        W1 = Wpre[l][:, :D]
        W2 = Wpre[l][:, D:]
        Wh = Wpost[l][:, :D]
        Wid = Wpost[l][:, D:5 * D]
        Wamp = Wpost[l][:, 5 * D:9 * D]
        Watt = Wpost[l][:, 9 * D:13 * D]
        Weff = Wlin[l] @ (Wid + k1 * Wamp + k2 * Watt)
        W0 = Weff[:, :D]
        W1b = Weff[:, D:2 * D]
        W2b = Weff[:, 2 * D:3 * D]
        W3b = Weff[:, 3 * D:]
        Whh = Wlin[l] @ Wh + (W0 + W1b + W2b) @ W1
        wpack[:, l, 0, :] = W2.T
        wpack[:, l, 1, :] = Whh.T
        wpack[:, l, 2, :] = (W0 / DEG).T
        wpack[:, l, 3, :] = W1b.T
        wpack[:, l, 4, :] = W2b.T
        wpack[:, l, 5, :] = W3b.T
        bpack[:, l] = ((W0 + W1b + W2b) @ bpre[l] + Wlin[l] @ bpost[l] + blin[l])
    wpack16 = wpack.astype(ml_dtypes.bfloat16)

    in_maps = []
    for c in range(CORES):
        xs = x[c * NC:(c + 1) * NC]
        xT = np.zeros((D, NP), np.float32)
        xT[:, :NC] = xs.T
        cols = col[c * NC * DEG:(c + 1) * NC * DEG]
        cols = np.concatenate([cols, np.zeros(((NP - NC) * DEG,), np.int64)])
        s = cols // NC
        o = cols % NC
        # invalid-half positions point at a pad row (B==0) of the view
        lo = np.where(s < LOB, s * NP + o, NC)
        hi = np.where(s >= LOB, (s - LOB) * NP + o, NC)
        assert lo.max() < 32768 and hi.max() < 32768
        # edge order: position i = (tile_in_chunk*128 + node)*12 + k, wrapped
        # into 16 partitions: partition i%16, slot i//16, per chunk
        idxa = np.zeros((16, CHUNKS, 2, W), np.int16)
        for ch in range(CHUNKS):
            tl = min(CT, TILES - ch * CT)
            e0 = ch * CT * 128 * DEG
            n = tl * 128 * DEG
            pos = np.arange(n)
            for j, arr in enumerate((lo, hi)):
                vals = arr[e0:e0 + n].astype(np.int16)
                idxa[pos % 16, ch, j, pos // 16] = vals
        # the 16-partition wrapped block must be replicated to all 128
        # partitions (one copy per Q7 core)
        idxa = np.tile(idxa, (8, 1, 1, 1))
        in_maps.append({
            "xT": xT,
            "idxw": idxa,
            "wpack": wpack16,
            "bpack": bpack,
        })
    return in_maps


_CACHE = {}


def kernel(x, edge_index, Wpre, bpre, Wpost, bpost, Wlin, blin):
    cfg = Cfg(np.asarray(x).shape[0])
    in_maps = prep_inputs(cfg, x, edge_index, Wpre, bpre, Wpost, bpost, Wlin, blin)
    if cfg.N not in _CACHE:
        _CACHE[cfg.N] = build(cfg)
    nc = _CACHE[cfg.N]
    res = run_bass_kernel_spmd(nc, in_maps, list(range(CORES)))
    outs = []
    for c in range(CORES):
        oT = res.results[c]["outT"]
        outs.append(np.ascontiguousarray(oT[:, :cfg.NC].T))
    return np.concatenate(outs, axis=0).astype(np.float32)


# revision 7
# speedup vs baseline: 2.0618x; 2.0618x over previous
"""Trainium2 Bass kernel for ConditionedPNA (3-layer PNAConv, N=50000, D=128, DEG=12).

Sharding/strategy (8 NeuronCores, SPMD):
  - Nodes sharded N/8 per core (padded to a multiple of 128). edge_index row is
    sorted (repeat(arange(N), DEG)), so each node's DEG edges are contiguous and
    colocated with the node's core; segment reductions are purely local.
  - Algebra: m_e = A[row_e] + B[col_e] + bpre with A = h @ Wpre[:, :D].T,
    B = h @ Wpre[:, D:].T. All aggregators reduce to segment stats of B[col]:
    mean = C + S/12, max = C + MX, min = C + MN,
    std = sqrt(relu(S2/12 - (S/12)^2) + 1e-5) (the C = A + bpre term cancels).
    deg == DEG everywhere -> degree scalers are constants folded into Wpost;
    Wlin, bpost, blin and the residual are folded in on the host.
  - Per layer: B shard (node-major, f32) -> AllGather -> full table in DRAM;
    per 128-node tile, 12 indirect DMAs (one per edge slot, 128 rows each,
    spread over 4 SWDGE queues) gather B[col] into a (128 nodes, 12, 128) tile;
    contiguous tt-trees reduce S/MX/MN/S2; agg chain; PE transposes the four
    agg parts to feature-major; 5-matmul post-GEMM accumulates
    h'^T = WhhT.T@h^T + sum_j WeffT_j.T@part_j^T (+bias) with the residual
    inside Whh. Full f32 throughout.
"""
import math
import numpy as np

import concourse.bass as bass
from concourse import bacc
import concourse.tile as tile
from concourse import mybir
from concourse.masks import make_identity
from concourse.bass_utils import run_bass_kernel_spmd

D, DEG, L, CORES = 128, 12, 3, 8
NQ = 4  # SWDGE queues
F32 = mybir.dt.float32
I32 = mybir.dt.int32

_hist = np.array([1.0] * 10 + [2.0] * 10)
AVG_LOG = float((np.log(np.arange(20) + 1.0) * _hist).sum() / _hist.sum())


class Cfg:
    def __init__(self, n):
        self.N = n
        self.NC = n // CORES
        self.NP = ((self.NC + 127) // 128) * 128
        self.TILES = self.NP // 128
        self.AG_ROWS = CORES * self.NP


def _tree(nc, eng, g3, work, out, opname):
    """g3: (128, 12, 128) AP (k-major free); work: (128, 6, 128); out f32 (128,128)."""
    A = mybir.AluOpType
    op = {"add": A.add, "max": A.max, "min": A.min}[opname]
    eng.tensor_tensor(out=work[:, 0:6, :], in0=g3[:, 0:6, :], in1=g3[:, 6:12, :], op=op)
    eng.tensor_tensor(out=work[:, 0:3, :], in0=work[:, 0:3, :], in1=work[:, 3:6, :], op=op)
    eng.tensor_tensor(out=work[:, 0:1, :], in0=work[:, 0:1, :], in1=work[:, 2:3, :], op=op)
    eng.tensor_tensor(out=out[:], in0=work[:, 0, :], in1=work[:, 1, :], op=op)


def build(cfg, repeat=1, ablate="FULL"):
    A = mybir.AluOpType
    AF = mybir.ActivationFunctionType
    NP, TILES = cfg.NP, cfg.TILES
    nc = bacc.Bacc("TRN2", target_bir_lowering=False, num_devices=CORES,
                   num_swdge_queues=NQ)

    xT = nc.dram_tensor("xT", [D, NP], F32, kind="ExternalInput")
    idx = nc.dram_tensor("idx", [TILES, 128, DEG], I32, kind="ExternalInput")
    wpack = nc.dram_tensor("wpack", [D, L, 7, D], F32, kind="ExternalInput")
    bpack = nc.dram_tensor("bpack", [D, L, 2], F32, kind="ExternalInput")
    outT = nc.dram_tensor("outT", [D, NP], F32, kind="ExternalOutput")

    agin = nc.dram_tensor("agin", [NP, D], F32)
    agout = nc.dram_tensor("agout", [cfg.AG_ROWS, D], F32, addr_space="Shared")

    qi = 0
    with tile.TileContext(nc) as tc:
        with (
            tc.tile_pool(name="persist", bufs=1) as pp,
            tc.tile_pool(name="gat", bufs=3) as gp,
            tc.tile_pool(name="work", bufs=2) as wp,
            tc.tile_pool(name="small", bufs=2) as sp,
            tc.tile_pool(name="psum", bufs=2, space="PSUM") as ps,
        ):
            hT_a = pp.tile([D, NP], F32)
            hT_b = pp.tile([D, NP], F32)
            hT = [hT_a, hT_b]
            W = pp.tile([D, L, 7, D], F32)
            nc.sync.dma_start(out=W[:], in_=wpack[:])
            B = pp.tile([D, L, 2], F32)
            nc.sync.dma_start(out=B[:], in_=bpack[:])
            eps = pp.tile([D, 1], F32)
            nc.vector.memset(eps[:], 1e-5)
            ident = pp.tile([D, D], F32)
            make_identity(nc, ident[:])
            nc.sync.dma_start(out=hT[0][:], in_=xT[:])

            for ll in range(repeat * L):
                l = ll % L
                hin = hT[ll % 2]
                hout = hT[(ll + 1) % 2]
                w1t = W[:, l, 0, :]
                w2t = W[:, l, 1, :]
                whh = W[:, l, 2, :]
                weff = [W[:, l, 3 + j, :] for j in range(4)]
                bout = B[:, l, 1:2]

                # ---- B shard (node-major) -> agin -> AllGather ----
                for t in range(TILES):
                    bp = ps.tile([128, D], F32, space="PSUM", tag="bp")
                    nc.tensor.matmul(
                        out=bp[:], lhsT=hin[:, t * 128:(t + 1) * 128], rhs=w2t,
                        start=True, stop=True,
                    )
                    bs = sp.tile([128, D], F32, tag="bs")
                    nc.scalar.activation(bs[:], bp[:], AF.Copy)
                    nc.sync.dma_start(out=agin[t * 128:(t + 1) * 128, :], in_=bs[:])
                nc.gpsimd.collective_compute(
                    "AllGather", A.bypass,
                    replica_groups=[list(range(CORES))],
                    ins=[agin[:]], outs=[agout[:]],
                )

                # ---- per 128-node tile ----
                for t in range(TILES):
                    hslab = hin[:, t * 128:(t + 1) * 128]
                    itile = sp.tile([128, DEG], I32, tag="itile")
                    nc.sync.dma_start(out=itile[:], in_=idx[t])
                    gm = gp.tile([128, DEG, D], F32, tag="gm")
                    for k in range(DEG):
                        inst = nc.gpsimd.indirect_dma_start(
                            out=gm[:, k, :], out_offset=None, in_=agout[:],
                            in_offset=bass.IndirectOffsetOnAxis(ap=itile[:, k:k + 1], axis=0),
                        )
                        if NQ > 1:
                            inst.ins.queue = f"qPoolDynamic{(qi % NQ) or ''}"
                            qi += 1

                    # C = A (node-major); bpre is folded into bout on the host
                    apsum = ps.tile([128, D], F32, space="PSUM", tag="apsum")
                    nc.tensor.matmul(out=apsum[:], lhsT=hslab, rhs=w1t, start=True, stop=True)
                    C = sp.tile([128, D], F32, tag="C")
                    nc.scalar.activation(C[:], apsum[:], AF.Copy)

                    if ablate == "G":
                        S = sp.tile([128, D], F32, tag="S")
                        wS = wp.tile([128, 6, D], F32, tag="wS")
                        _tree(nc, nc.vector, gm[:], wS, S[:], "add")
                        hp = ps.tile([128, 128], F32, space="PSUM", tag="hp")
                        nc.tensor.matmul(out=hp[:], lhsT=whh, rhs=hslab, start=True, stop=True)
                        nc.scalar.activation(
                            hout[:, t * 128:(t + 1) * 128], hp[:], AF.Identity, bias=bout)
                        continue
                    g2 = gp.tile([128, DEG, D], F32, tag="g2")
                    nc.scalar.square(g2[:], gm[:])
                    wS = wp.tile([128, 6, D], F32, tag="wS")
                    wX = wp.tile([128, 6, D], F32, tag="wX")
                    wN = wp.tile([128, 6, D], F32, tag="wN")
                    w2b = wp.tile([128, 6, D], F32, tag="w2b")
                    S = sp.tile([128, D], F32, tag="S")
                    MX = sp.tile([128, D], F32, tag="MX")
                    MN = sp.tile([128, D], F32, tag="MN")
                    S2 = sp.tile([128, D], F32, tag="S2")
                    _tree(nc, nc.vector, gm[:], wS, S[:], "add")
                    _tree(nc, nc.vector, gm[:], wX, MX[:], "max")
                    _tree(nc, nc.vector, gm[:], wN, MN[:], "min")
                    _tree(nc, nc.vector, g2[:], w2b, S2[:], "add")

                    if ablate == "GT":
                        hp = ps.tile([128, 128], F32, space="PSUM", tag="hp")
                        nc.tensor.matmul(out=hp[:], lhsT=whh, rhs=hslab, start=True, stop=True)
                        nc.scalar.activation(
                            hout[:, t * 128:(t + 1) * 128], hp[:], AF.Identity, bias=bout)
                        continue
                    meanB = sp.tile([128, D], F32, tag="meanB")
                    nc.vector.tensor_scalar_mul(meanB[:], S[:], 1.0 / DEG)
                    mean = sp.tile([128, D], F32, tag="mean")
                    nc.vector.tensor_tensor(out=mean[:], in0=meanB[:], in1=C[:], op=A.add)
                    mxc = sp.tile([128, D], F32, tag="mxc")
                    nc.vector.tensor_tensor(out=mxc[:], in0=MX[:], in1=C[:], op=A.add)
                    mnc = sp.tile([128, D], F32, tag="mnc")
                    nc.vector.tensor_tensor(out=mnc[:], in0=MN[:], in1=C[:], op=A.add)
                    m2 = sp.tile([128, D], F32, tag="m2")
                    nc.vector.tensor_tensor(out=m2[:], in0=meanB[:], in1=meanB[:], op=A.mult)
                    var = sp.tile([128, D], F32, tag="var")
                    nc.vector.scalar_tensor_tensor(
                        out=var[:], in0=S2[:], scalar=1.0 / DEG, in1=m2[:],
                        op0=A.mult, op1=A.subtract,
                    )
                    varc = sp.tile([128, D], F32, tag="varc")
                    nc.vector.tensor_scalar_max(varc[:], var[:], 0.0)
                    std = sp.tile([128, D], F32, tag="std")
                    nc.scalar.activation(std[:], varc[:], AF.Sqrt, bias=eps[:])

                    # transpose agg parts to feature-major and post-GEMM
                    hp = ps.tile([128, 128], F32, space="PSUM", tag="hp")
                    nc.tensor.matmul(out=hp[:], lhsT=whh, rhs=hslab, start=True, stop=False)
                    for j, part in enumerate([mean, mxc, mnc, std]):
                        ptp = ps.tile([128, 128], F32, space="PSUM", tag="tp")
                        nc.tensor.transpose(out=ptp[:], in_=part[:], identity=ident[:])
                        pts = sp.tile([128, 128], F32, tag="pts")
                        if j % 2 == 0:
                            nc.scalar.activation(pts[:], ptp[:], AF.Copy)
                        else:
                            nc.vector.tensor_copy(pts[:], ptp[:])
                        nc.tensor.matmul(
                            out=hp[:], lhsT=weff[j], rhs=pts[:],
                            start=False, stop=(j == 3),
                        )
                    nc.scalar.activation(
                        hout[:, t * 128:(t + 1) * 128], hp[:], AF.Identity, bias=bout,
                    )
                if cfg.NC < NP:
                    nc.vector.memset(hout[:, cfg.NC:NP], 0.0)

            nc.sync.dma_start(out=outT[:], in_=hT[(repeat * L) % 2][:])
    nc.compile()
    return nc


def prep_inputs(cfg, x, edge_index, Wpre, bpre, Wpost, bpost, Wlin, blin):
    x = np.asarray(x, np.float32)
    ei = np.asarray(edge_index)
    Wpre = np.asarray(Wpre, np.float32)
    bpre = np.asarray(bpre, np.float32)
    Wpost = np.asarray(Wpost, np.float32)
    bpost = np.asarray(bpost, np.float32)
    Wlin = np.asarray(Wlin, np.float32)
    blin = np.asarray(blin, np.float32)
    N, NC, NP, TILES = cfg.N, cfg.NC, cfg.NP, cfg.TILES

    row = ei[0].astype(np.int64)
    col = ei[1].astype(np.int64)
    assert (row == np.repeat(np.arange(N), DEG)).all(), "kernel assumes sorted rows, uniform degree"
    dlog = math.log(DEG + 1.0)
    k1 = dlog / AVG_LOG
    k2 = AVG_LOG / dlog

    wpack = np.zeros((D, L, 7, D), np.float32)
    bpack = np.zeros((D, L, 2), np.float32)
    I = np.eye(D, dtype=np.float32)
    for l in range(L):
        W1 = Wpre[l][:, :D]
        W2 = Wpre[l][:, D:]
        Wh = Wpost[l][:, :D]
        Wid = Wpost[l][:, D:5 * D]
        Wamp = Wpost[l][:, 5 * D:9 * D]
        Watt = Wpost[l][:, 9 * D:13 * D]
        Weff = Wlin[l] @ (Wid + k1 * Wamp + k2 * Watt)
        Whh = Wlin[l] @ Wh + I
        wpack[:, l, 0, :] = W1.T
        wpack[:, l, 1, :] = W2.T
        wpack[:, l, 2, :] = Whh.T
        for j in range(4):
            wpack[:, l, 3 + j, :] = Weff[:, j * D:(j + 1) * D].T
        bpack[:, l, 0] = bpre[l]
        bpack[:, l, 1] = (Wlin[l] @ bpost[l] + blin[l]
                          + (Weff[:, :D] + Weff[:, D:2*D] + Weff[:, 2*D:3*D]) @ bpre[l])

    in_maps = []
    for c in range(CORES):
        xs = x[c * NC:(c + 1) * NC]
        xT = np.zeros((D, NP), np.float32)
        xT[:, :NC] = xs.T
        cols = col[c * NC * DEG:(c + 1) * NC * DEG]
        cols = np.concatenate([cols, np.zeros(((NP - NC) * DEG,), np.int64)])
        cols = cols.reshape(NP, DEG)
        gr = (cols // NC) * NP + (cols % NC)
        idxa = gr.reshape(TILES, 128, DEG).astype(np.int32)
        in_maps.append({
            "xT": xT,
            "idx": idxa,
            "wpack": wpack,
            "bpack": bpack,
        })
    return in_maps


_CACHE = {}


def kernel(x, edge_index, Wpre, bpre, Wpost, bpost, Wlin, blin):
    cfg = Cfg(np.asarray(x).shape[0])
    in_maps = prep_inputs(cfg, x, edge_index, Wpre, bpre, Wpost, bpost, Wlin, blin)
    if cfg.N not in _CACHE:
        _CACHE[cfg.N] = build(cfg)
    nc = _CACHE[cfg.N]
    res = run_bass_kernel_spmd(nc, in_maps, list(range(CORES)))
    outs = []
    for c in range(CORES):
        oT = res.results[c]["outT"]
        outs.append(np.ascontiguousarray(oT[:, :cfg.NC].T))
    return np.concatenate(outs, axis=0).astype(np.float32)
